# revision 1
# baseline (speedup 1.0000x reference)
"""Trainium2 Bass kernel for a GPT-style transformer block (pre-LN attention +
FFN), data-parallel over the batch axis across 8 NeuronCores.

Reference semantics (B=2048, T=64, C=384, H=6, HS=64, DFF=1536):
    h  = LN(x; ln1) ; q,k,v = h @ Wq/Wk/Wv (per head)
    S  = q k^T (no 1/sqrt(d) scale), causal mask, softmax over the QUERY axis
    o  = (softmax S) v ; x2 = x + o @ Wo + bo
    f  = relu(LN(x2; ln2) @ W1 + b1) @ W2 + b2 ; out = x2 + f

Layout strategy per 128-token tile (= 2 batch items):
  - Residual stream token-major (tokens on SBUF partitions) -> LayerNorm via
    bn_stats over the free axis; LN affine folded into the weights host-side.
  - Post-LN activations transposed to feature-major via DMA-xbar transpose
    (bf16), so projections run with the weights as the stationary operand.
  - q,k produced feature-major; S^T = k q^T per (item, head) so the
    reference's query-axis softmax becomes a free-axis softmax; v produced
    token-major, o accumulated feature-major, projections bring it back.
  - bf16 matmul operands, fp32 PSUM accumulation, fp32 residual stream.
"""

import numpy as np
import ml_dtypes

import concourse.bass as bass
import concourse.mybir as mybir
from concourse.bass_utils import run_bass_kernel_spmd
from concourse.tile import TileContext

F32 = mybir.dt.float32
BF16 = mybir.dt.bfloat16
AF = mybir.ActivationFunctionType
ALU = mybir.AluOpType

B, T, C, H, HS = 2048, 64, 384, 6, 64
DFF = 4 * C
EPS = 1e-5
N_CORES = 8
P = 128               # SBUF partitions / tokens per tile
ITEMS_PER_TILE = P // T   # 2
KC = C // P           # 3 contraction chunks of 128 over C
MC_FF = DFF // P      # 12 chunks over DFF
NEG = -1.0e30

_ctr = [0]


def _split_sync_waits(nc, max_waits=1):
    """This walrus build rejects instructions with more than one sync-wait
    command. Keep one wait per instruction; hoist the rest onto same-engine
    NoOps inserted immediately before it (same blocking semantics)."""
    for f in nc.m.functions:
        for bb in f.blocks:
            insts = bb.instructions
            if not any(
                i.sync_info is not None and len(i.sync_info.on_wait) > max_waits
                for i in insts
            ):
                continue
            new = []
            for inst in insts:
                si = inst.sync_info
                if si is not None and len(si.on_wait) > max_waits:
                    waits = list(si.on_wait)
                    for w in waits[:-max_waits]:
                        _ctr[0] += 1
                        nop = mybir.InstNoOp(
                            name=f"WS-{_ctr[0]}",
                            engine=inst.engine,
                            ins=[],
                            outs=[],
                            sync_info=mybir.SyncInfo(on_wait=[w], on_update=[]),
                        )
                        nc.register_instruction(nop)
                        new.append(nop)
                    inst.sync_info = mybir.SyncInfo(
                        on_wait=waits[-max_waits:], on_update=list(si.on_update)
                    )
                new.append(inst)
            bb.instructions = new


def build_program(n_items, unroll=4, flags=(), reps=1):
    """Build the SPMD Bass program for one core processing `n_items` batch
    items. `flags` is a tuple of bias-path names that are non-zero and need
    device-side adds ('qb', 'kb', 'vb', 'ob', 'b1', 'b2'). `reps` repeats the
    whole workload (for wall-clock differencing benchmarks)."""
    flags = set(flags)
    n_tiles = n_items * T // P
    assert n_items * T % P == 0 and n_tiles % unroll == 0

    nc = bass.Bass()
    xs = nc.declare_dram_parameter("xs", [n_items, T, C], F32, isOutput=False)
    out = nc.declare_dram_parameter("out", [n_items, T, C], F32, isOutput=True)
    wq = nc.declare_dram_parameter("wq", [C, C], BF16, isOutput=False)
    wk = nc.declare_dram_parameter("wk", [C, C], BF16, isOutput=False)
    wv = nc.declare_dram_parameter("wv", [C, C], BF16, isOutput=False)
    wo = nc.declare_dram_parameter("wo", [C, C], BF16, isOutput=False)
    w1 = nc.declare_dram_parameter("w1", [C, DFF], BF16, isOutput=False)
    w2 = nc.declare_dram_parameter("w2", [DFF, C], BF16, isOutput=False)
    mask = nc.declare_dram_parameter("mask", [P, T], BF16, isOutput=False)
    biases = {}
    for nm, dim in (("qb", KC), ("kb", KC), ("b1", MC_FF)):
        if nm in flags:
            biases[nm] = nc.declare_dram_parameter(nm, [P, dim], F32, isOutput=False)
    for nm in ("vb", "ob", "b2"):
        if nm in flags:
            biases[nm] = nc.declare_dram_parameter(nm, [C], F32, isOutput=False)

    x4 = (xs[:].rearrange("b t c -> (b t) c")
          .rearrange("(n u p) c -> n u p c", u=unroll, p=P))
    o4 = (out[:].rearrange("b t c -> (b t) c")
          .rearrange("(n u p) c -> n u p c", u=unroll, p=P))

    with TileContext(nc) as tc:
        with (
            tc.tile_pool(name="const", bufs=1) as const,
            tc.tile_pool(name="io", bufs=3) as io,
            tc.tile_pool(name="act", bufs=2) as act,
            tc.tile_pool(name="qko", bufs=2) as qko,
            tc.tile_pool(name="sm", bufs=3) as sm,
            tc.tile_pool(name="ffn", bufs=2) as ffn,
            tc.tile_pool(name="small", bufs=4) as small,
            tc.tile_pool(name="ps_qk", bufs=2, space="PSUM") as ps_qk,
            tc.tile_pool(name="ps_att", bufs=2, space="PSUM") as ps_att,
            tc.tile_pool(name="ps_v", bufs=1, space="PSUM") as ps_v,
            tc.tile_pool(name="ps_pr", bufs=1, space="PSUM") as ps_pr,
            tc.tile_pool(name="ps_f2", bufs=1, space="PSUM") as ps_f2,
            tc.tile_pool(name="ps_f1", bufs=1, space="PSUM") as ps_f1,
        ):
            # ---- load constants into SBUF once ----
            wq_sb = [const.tile([P, C], BF16, tag=f"wq{i}", name=f"wq{i}") for i in range(KC)]
            wk_sb = [const.tile([P, C], BF16, tag=f"wk{i}", name=f"wk{i}") for i in range(KC)]
            wv_sb = [const.tile([P, C], BF16, tag=f"wv{i}", name=f"wv{i}") for i in range(KC)]
            wo_sb = [const.tile([P, C], BF16, tag=f"wo{i}", name=f"wo{i}") for i in range(KC)]
            w1_sb = [const.tile([P, DFF], BF16, tag=f"w1{i}", name=f"w1{i}") for i in range(KC)]
            w2_sb = [const.tile([P, C], BF16, tag=f"w2{i}", name=f"w2{i}") for i in range(MC_FF)]
            for i in range(KC):
                nc.sync.dma_start(out=wq_sb[i], in_=wq[i * P:(i + 1) * P, :])
                nc.sync.dma_start(out=wk_sb[i], in_=wk[i * P:(i + 1) * P, :])
                nc.sync.dma_start(out=wv_sb[i], in_=wv[i * P:(i + 1) * P, :])
                nc.sync.dma_start(out=wo_sb[i], in_=wo[i * P:(i + 1) * P, :])
                nc.sync.dma_start(out=w1_sb[i], in_=w1[i * P:(i + 1) * P, :])
            for i in range(MC_FF):
                nc.sync.dma_start(out=w2_sb[i], in_=w2[i * P:(i + 1) * P, :])
            mask_sb = const.tile([P, T], BF16, tag="mask", name="mask")
            nc.sync.dma_start(out=mask_sb, in_=mask[:, :])
            bias_sb = {}
            for nm in ("qb", "kb", "b1"):
                if nm in flags:
                    t = const.tile(list(biases[nm].shape), F32, tag=nm)
                    nc.sync.dma_start(out=t, in_=biases[nm][:, :])
                    bias_sb[nm] = t
            for nm in ("vb", "ob", "b2"):
                if nm in flags:
                    t = const.tile([P, C], F32, tag=nm)
                    ap = biases[nm][:]
                    rep = bass.AP(tensor=ap.tensor, offset=ap.offset,
                                  ap=[[0, P]] + list(ap.ap))
                    nc.sync.dma_start(out=t, in_=rep)
                    bias_sb[nm] = t

            eps_sb = const.tile([P, 1], F32, tag="eps", name="eps")
            nc.vector.memset(eps_sb, EPS)

            def layer_norm(x_in, tag):
                """token-major LN -> bf16 normalized output (affine folded
                into the weights on the host)."""
                st6 = small.tile([P, 6], F32, tag=f"st6_{tag}", name=f"st6_{tag}")
                nc.vector.bn_stats(st6, x_in)
                mv = small.tile([P, 2], F32, tag=f"mv_{tag}", name=f"mv_{tag}")
                nc.vector.bn_aggr(mv, st6)
                std = small.tile([P, 1], F32, tag=f"std_{tag}", name=f"std_{tag}")
                nc.scalar.activation(std, mv[:, 1:2], AF.Sqrt, bias=eps_sb)
                rstd = small.tile([P, 1], F32, tag=f"rstd_{tag}", name=f"rstd_{tag}")
                nc.vector.reciprocal(rstd, std)
                h = act.tile([P, C], BF16, tag=f"h_{tag}", name=f"h_{tag}")
                nc.vector.tensor_scalar(h, x_in, mv[:, 0:1], rstd,
                                        ALU.subtract, ALU.mult)
                return h

            def transpose3(h, tag):
                """[128 tok, 384] bf16 -> 3x [128 feat, 128 tok] via DMA xbar."""
                outs = []
                for c in range(KC):
                    hf = act.tile([P, P], BF16, tag=f"{tag}{c}", name=f"{tag}{c}")
                    nc.sync.dma_start_transpose(out=hf, in_=h[:, c * P:(c + 1) * P])
                    outs.append(hf)
                return outs

            def group_load(g):
                xg = io.tile([P, unroll, C], F32, tag="xg", name="xg")
                nc.sync.dma_start(out=xg, in_=x4[g].rearrange("u p c -> p u c"))
                og = io.tile([P, unroll, C], F32, tag="og", name="og")
                return xg, og

            def group_store(g, og):
                nc.sync.dma_start(out=o4[g].rearrange("u p c -> p u c"), in_=og)

            def tile_body(xg, og, j):
                x_t = xg[:, j, :]

                # ---- LN1 + transpose ----
                h = layer_norm(x_t, "ln1")
                h_fm = transpose3(h, "hfm")

                # ---- q,k feature-major / v token-major ----
                # q/k: one [128, 3*128] PSUM bank each; feature-chunk mc's
                # 128 token columns live at free cols mc*128..  (all matmuls
                # use the full 128-row PE array -> same bank is legal).
                qk_sb = []
                for w_sb, b_nm in ((wq_sb, "qb"), (wk_sb, "kb")):
                    ps = ps_qk.tile([P, C], F32, tag="qk", name="qk")
                    for mc in range(KC):
                        for kc in range(KC):
                            nc.tensor.matmul(
                                ps[:, mc * P:(mc + 1) * P],
                                lhsT=w_sb[kc][:, mc * P:(mc + 1) * P],
                                rhs=h_fm[kc], start=(kc == 0), stop=(kc == KC - 1))
                    sb = qko.tile([P, C], BF16, tag=f"{b_nm}sb", name=f"{b_nm}sb")
                    if b_nm in flags:
                        for mc in range(KC):
                            nc.scalar.activation(sb[:, mc * P:(mc + 1) * P],
                                                 ps[:, mc * P:(mc + 1) * P],
                                                 AF.Identity,
                                                 bias=bias_sb[b_nm][:, mc:mc + 1])
                    else:
                        nc.vector.tensor_copy(sb, ps)
                    qk_sb.append(sb)
                q_sb, k_sb = qk_sb
                v_ps = ps_v.tile([P, C], F32, tag="v", name="v")
                for kc in range(KC):
                    nc.tensor.matmul(v_ps, lhsT=h_fm[kc], rhs=wv_sb[kc],
                                     start=(kc == 0), stop=(kc == KC - 1))
                v_sb = act.tile([P, C], BF16, tag="v", name="v")
                if "vb" in flags:
                    nc.vector.tensor_tensor(out=v_sb, in0=v_ps,
                                            in1=bias_sb["vb"], op=ALU.add)
                else:
                    nc.vector.tensor_copy(v_sb, v_ps)

                # ---- attention ----
                # HW: matmuls sharing a PSUM bank must share a PE row-group.
                # S^T banks: one per head-parity (3 heads x 2 items each, all
                # K-partitions hh*64..), softmax ops run on [128,192] batches.
                pts = []
                for hh in range(2):
                    st = ps_att.tile([P, KC * T], F32, tag="att", name="att")
                    for hp in range(KC):
                        for b in range(ITEMS_PER_TILE):
                            nc.tensor.matmul(
                                st[b * T:(b + 1) * T, hp * T:(hp + 1) * T],
                                lhsT=k_sb[hh * T:(hh + 1) * T,
                                          hp * P + b * T:hp * P + (b + 1) * T],
                                rhs=q_sb[hh * T:(hh + 1) * T,
                                         hp * P + b * T:hp * P + (b + 1) * T],
                                start=True, stop=True,
                                tile_position=(hh * T, b * T))
                    et = sm.tile([P, KC * T], BF16, tag="et", name="et")
                    nc.scalar.activation(et, st, AF.Exp)
                    masked = sm.tile([P, KC * T], BF16, tag="masked", name="masked")
                    m_b = bass.AP(tensor=mask_sb.tensor, offset=mask_sb.offset,
                                  ap=[list(mask_sb.ap[0]), [0, KC],
                                      list(mask_sb.ap[1])])
                    et3 = et.rearrange("p (k t) -> p k t", k=KC)
                    nc.gpsimd.tensor_tensor(
                        out=masked.rearrange("p (k t) -> p k t", k=KC),
                        in0=et3, in1=m_b, op=ALU.mult)
                    sums = small.tile([P, KC], F32, tag="sums", name="sums")
                    nc.vector.reduce_sum(
                        out=sums, in_=masked.rearrange("p (k t) -> p k t", k=KC),
                        axis=mybir.AxisListType.X)
                    rec = small.tile([P, KC], F32, tag="rec", name="rec")
                    nc.vector.reciprocal(rec, sums)
                    pt = sm.tile([P, KC * T], BF16, tag="pt", name="pt")
                    r_b = bass.AP(tensor=rec.tensor, offset=rec.offset,
                                  ap=[list(rec.ap[0]), list(rec.ap[1]), [0, T]])
                    nc.gpsimd.tensor_tensor(
                        out=pt.rearrange("p (k t) -> p k t", k=KC),
                        in0=masked.rearrange("p (k t) -> p k t", k=KC),
                        in1=r_b, op=ALU.mult)
                    pts.append(pt)
                    del et, masked, sums, rec, pt
                # o^T banks: one per item (row-group = item); head-pair hp's
                # 64 token-cols at free offset hp*64; copied into one
                # [128, 3*128] feature-major o with a single strided DVE copy.
                o_sb = qko.tile([P, C], BF16, tag="osb", name="osb")
                for b in range(ITEMS_PER_TILE):
                    o_ps = ps_att.tile([P, KC * T], F32, tag="att", name="att")
                    for hp in range(KC):
                        for hh in range(2):
                            head = 2 * hp + hh
                            nc.tensor.matmul(
                                o_ps[hh * T:(hh + 1) * T, hp * T:(hp + 1) * T],
                                lhsT=v_sb[b * T:(b + 1) * T,
                                          head * HS:(head + 1) * HS],
                                rhs=pts[hh][b * T:(b + 1) * T,
                                            hp * T:(hp + 1) * T],
                                start=True, stop=True,
                                tile_position=(b * T, hh * T))
                    o_view = bass.AP(tensor=o_sb.tensor,
                                     offset=o_sb.offset + b * T,
                                     ap=[list(o_sb.ap[0]), [P, KC], [1, T]])
                    nc.vector.tensor_copy(
                        o_view, o_ps.rearrange("p (k t) -> p k t", k=KC))

                # ---- output projection + residual ----
                pr_ps = ps_pr.tile([P, C], F32, tag="pr", name="pr")
                for hp in range(KC):
                    nc.tensor.matmul(pr_ps, lhsT=o_sb[:, hp * P:(hp + 1) * P],
                                     rhs=wo_sb[hp],
                                     start=(hp == 0), stop=(hp == KC - 1))
                x2 = io.tile([P, C], F32, tag="x2", name="x2")
                nc.vector.tensor_tensor(out=x2, in0=x_t, in1=pr_ps, op=ALU.add)
                if "ob" in flags:
                    nc.vector.tensor_tensor(out=x2, in0=x2, in1=bias_sb["ob"],
                                            op=ALU.add)

                # ---- LN2 + FFN ----
                h2 = layer_norm(x2, "ln2")
                h2_fm = transpose3(h2, "h2fm")
                f1_sb = []
                for fg in range(KC):  # 3 groups of 4 dff chunks
                    f1_ps = ps_f1.tile([P, 4 * P], F32, tag="f1", name="f1")
                    for j4 in range(4):
                        mc = 4 * fg + j4
                        for kc in range(KC):
                            nc.tensor.matmul(
                                f1_ps[:, j4 * P:(j4 + 1) * P],
                                lhsT=w1_sb[kc][:, mc * P:(mc + 1) * P],
                                rhs=h2_fm[kc], start=(kc == 0), stop=(kc == KC - 1))
                    fs = ffn.tile([P, 4 * P], BF16, tag=f"f1sb{fg}", name=f"f1sb{fg}")
                    if "b1" in flags:
                        for j4 in range(4):
                            mc = 4 * fg + j4
                            nc.scalar.activation(
                                fs[:, j4 * P:(j4 + 1) * P],
                                f1_ps[:, j4 * P:(j4 + 1) * P], AF.Relu,
                                bias=bias_sb["b1"][:, mc:mc + 1])
                    else:
                        nc.vector.tensor_scalar_max(fs, f1_ps, 0.0)
                    f1_sb.append(fs)
                f2_ps = ps_f2.tile([P, C], F32, tag="f2", name="f2")
                for kc12 in range(MC_FF):
                    fg2, j4 = divmod(kc12, 4)
                    nc.tensor.matmul(
                        f2_ps, lhsT=f1_sb[fg2][:, j4 * P:(j4 + 1) * P],
                        rhs=w2_sb[kc12], start=(kc12 == 0), stop=(kc12 == MC_FF - 1))
                o_t = og[:, j, :]
                nc.vector.tensor_tensor(out=o_t, in0=x2, in1=f2_ps, op=ALU.add)
                if "b2" in flags:
                    nc.vector.tensor_tensor(out=o_t, in0=o_t, in1=bias_sb["b2"],
                                            op=ALU.add)

            def group_body(g):
                xg, og = group_load(g)
                for j in range(unroll):
                    tile_body(xg, og, j)
                group_store(g, og)

            n_groups = n_tiles // unroll
            if n_groups == 1 and reps == 1:
                group_body(0)
            elif reps == 1:
                with tc.For_i(0, n_groups, 1,
                              hint_engines=(mybir.EngineType.PE,)) as g:
                    group_body(g)
            else:
                with tc.For_i(0, reps, 1) as _r:
                    with tc.For_i(0, n_groups, 1,
                                  hint_engines=(mybir.EngineType.PE,)) as g:
                        group_body(g)

    _split_sync_waits(nc)
    return nc


def prepare_weights(ln1_w, ln1_b, Wq, Wk, Wv, Wo, bo, ln2_w, ln2_b, W1, b1, W2, b2):
    """Fold LN affines into the projection weights (exact linear algebra) and
    cast to bf16; returns (weight arrays dict, nonzero-bias flags tuple)."""
    f32 = np.float32
    wq2 = np.ascontiguousarray(np.transpose(np.asarray(Wq, f32), (1, 0, 2)).reshape(C, C))
    wk2 = np.ascontiguousarray(np.transpose(np.asarray(Wk, f32), (1, 0, 2)).reshape(C, C))
    wv2 = np.ascontiguousarray(np.transpose(np.asarray(Wv, f32), (1, 0, 2)).reshape(C, C))
    ln1_w = np.asarray(ln1_w, f32)
    ln1_b = np.asarray(ln1_b, f32)
    ln2_w = np.asarray(ln2_w, f32)
    ln2_b = np.asarray(ln2_b, f32)
    W1 = np.asarray(W1, f32)
    qb, kb, vb = ln1_b @ wq2, ln1_b @ wk2, ln1_b @ wv2
    arrs = {
        "wq": ln1_w[:, None] * wq2,
        "wk": ln1_w[:, None] * wk2,
        "wv": ln1_w[:, None] * wv2,
        "wo": np.asarray(Wo, f32),
        "w1": ln2_w[:, None] * W1,
        "w2": np.asarray(W2, f32),
    }
    arrs = {k: v.astype(ml_dtypes.bfloat16) for k, v in arrs.items()}
    b1f = np.asarray(b1, f32) + ln2_b @ W1
    b2f = np.asarray(b2, f32)
    obf = np.asarray(bo, f32)
    flags = []
    if np.any(qb != 0):
        flags.append("qb")
        arrs["qb"] = np.ascontiguousarray(qb.reshape(KC, P).T)
    if np.any(kb != 0):
        flags.append("kb")
        arrs["kb"] = np.ascontiguousarray(kb.reshape(KC, P).T)
    if np.any(vb != 0):
        flags.append("vb")
        arrs["vb"] = vb
    if np.any(obf != 0):
        flags.append("ob")
        arrs["ob"] = obf
    if np.any(b1f != 0):
        flags.append("b1")
        arrs["b1"] = np.ascontiguousarray(b1f.reshape(MC_FF, P).T)
    if np.any(b2f != 0):
        flags.append("b2")
        arrs["b2"] = b2f

    # causal mask in S^T coordinates: row = s (key), col = t (query);
    # keep t >= s, both mod 64 within each (item, head) quadrant.
    sidx = np.arange(P)[:, None] % T
    tidx = np.arange(T)[None, :]
    arrs["mask"] = np.where(tidx >= sidx, 1.0, 0.0).astype(ml_dtypes.bfloat16)
    return arrs, tuple(flags)


_cache = {}


def _get_program(n_items, flags, unroll=4, reps=1):
    key = (n_items, flags, unroll, reps)
    if key not in _cache:
        _cache[key] = build_program(n_items, unroll=unroll, flags=flags, reps=reps)
    return _cache[key]


def run_sharded(x, weight_arrs, flags, trace=False, unroll=4, reps=1):
    n_items = x.shape[0] // N_CORES
    nc = _get_program(n_items, flags, unroll, reps)
    shards = np.split(np.asarray(x, np.float32), N_CORES, axis=0)
    in_maps = []
    for i in range(N_CORES):
        m = {"xs": shards[i]}
        m.update(weight_arrs)
        in_maps.append(m)
    res = run_bass_kernel_spmd(nc, in_maps, list(range(N_CORES)), trace=trace)
    out = np.concatenate([res.results[i]["out"] for i in range(N_CORES)], axis=0)
    return out, res


def kernel(x, ln1_w, ln1_b, Wq, Wk, Wv, Wo, bo, ln2_w, ln2_b, W1, b1, W2, b2):
    arrs, flags = prepare_weights(ln1_w, ln1_b, Wq, Wk, Wv, Wo, bo,
                                  ln2_w, ln2_b, W1, b1, W2, b2)
    out, _ = run_sharded(x, arrs, flags)
    return out

